# revision 1
# baseline (speedup 1.0000x reference)
"""MoE routing + expert FFN kernel for 8 Trainium2 NeuronCores.

Sharding: data-parallel routing (core g owns token group g) + expert-parallel
FFN (core e owns expert e); dispatch/combine are on-device AllToAlls.

Per-core pipeline (SPMD, core id = g = e):
  1. Router: logits = tokens[g] @ w_router (fp32 on PE), softmax-max ->
     gate, argmax mask, capacity positions via triangular-matmul cumsum.
  2. Dispatch: one-hot slot matrix (addr[t] == slot iota) on DVE, then a
     PE matmul gathers token rows into per-expert slot buffers
     xdisp[e*C+c]; AllToAll #1 per H-half (dropped tokens match no slot).
  3. Expert FFN (bf16, fp32 accum): hT = relu(w1.T @ x).T kept in SBUF,
     y = hT.T @ w2 streamed to DRAM per H-half.
  4. AllToAll #2 per H-half (overlaps the other half's matmuls), combine:
     indirect-DMA gather by slot address (dropped tokens hit a zeroed
     dump row), scale by gate*kept, write out[g].
"""

import sys

sys.path.insert(0, "/opt/trn_rl_repo")

import numpy as np
import ml_dtypes

G, T, H, E, DFF, CAP = 8, 1024, 1024, 8, 4096, 128
NCORES = 8
P = 128

_STATE = {}


def _build_nc(fake_collectives=False, stages=None):
    from concourse import bacc
    import concourse.bass as bass
    import concourse.mybir as mybir
    import concourse.tile as tile

    f32 = mybir.dt.float32
    bf16 = mybir.dt.bfloat16
    i32 = mybir.dt.int32
    X = mybir.AxisListType.X
    AF = mybir.ActivationFunctionType
    OP = mybir.AluOpType

    nc = bacc.Bacc("TRN2", target_bir_lowering=False, debug=False,
                   num_devices=NCORES)

    tok_t = nc.dram_tensor("tok_t", [H, T], f32, kind="ExternalInput")
    tok_bf = nc.dram_tensor("tok_bf", [T, H], bf16, kind="ExternalInput")
    wr = nc.dram_tensor("wr", [H, E], f32, kind="ExternalInput")
    w1 = nc.dram_tensor("w1", [H, DFF], bf16, kind="ExternalInput")
    w2 = nc.dram_tensor("w2", [DFF, H], bf16, kind="ExternalInput")
    ones_c = nc.dram_tensor("ones_c", [P, P], bf16, kind="ExternalInput")
    utri_c = nc.dram_tensor("utri_c", [P, P], bf16, kind="ExternalInput")
    iota8 = nc.dram_tensor("iota8", [P, E], f32, kind="ExternalInput")
    siota = nc.dram_tensor("siota", [P, T], f32, kind="ExternalInput")
    out = nc.dram_tensor("out", [T, H], f32, kind="ExternalOutput")

    # Internal DRAM, split into H-column halves so each AllToAll can fire
    # as soon as its half is produced (overlapping the other half's
    # compute). ycomb halves carry an extra dump row (index T) that
    # dropped tokens gather zeros from.
    HH = H // 2
    xdisp = [nc.dram_tensor(f"xdisp{i}", [T, HH], bf16) for i in range(2)]
    xrecv = [nc.dram_tensor(f"xrecv{i}", [T, HH], bf16) for i in range(2)]
    yy = [nc.dram_tensor(f"yy{i}", [T, HH], f32) for i in range(2)]
    ycomb = [nc.dram_tensor(f"ycomb{i}", [T + 1, HH], f32) for i in range(2)]

    NT = T // P  # 8 token tiles per group
    RG = [list(range(NCORES))]
    ALL = {"router", "cumsum", "dispatch", "transpose", "m1", "m2", "combine"}
    stg = ALL if stages is None else set(stages)
    def _n(stage, n):
        return n if stage in stg else 0

    with tile.TileContext(nc) as tc:
        with (
            tc.tile_pool(name="const", bufs=1) as constp,
            tc.tile_pool(name="big", bufs=1) as big,
            tc.tile_pool(name="rt", bufs=4) as rtp,
            tc.tile_pool(name="w1s_p", bufs=2) as w1p,
            tc.tile_pool(name="w2s_p", bufs=2) as w2p,
            tc.tile_pool(name="io", bufs=4) as iop,
            tc.tile_pool(name="psr", bufs=1, space="PSUM") as psr,
            tc.tile_pool(name="ps1", bufs=2, space="PSUM") as ps1,
            tc.tile_pool(name="ps2", bufs=1, space="PSUM") as ps2,
        ):
            # ---- constants / small staging
            ones_sb = constp.tile([P, P], bf16)
            nc.sync.dma_start(ones_sb[:], ones_c[:, :])
            utri_sb = constp.tile([P, P], bf16)
            nc.sync.dma_start(utri_sb[:], utri_c[:, :])
            iota_sb = constp.tile([P, E], f32)
            nc.sync.dma_start(iota_sb[:], iota8[:, :])
            siota_sb = constp.tile([P, T], f32)
            nc.sync.dma_start(siota_sb[:], siota[:, :])
            wr_sb = constp.tile([P, E * 8], f32)
            for k in range(8):
                nc.sync.dma_start(wr_sb[:, k * E:(k + 1) * E],
                                  wr[k * P:(k + 1) * P, :])
            zrow = constp.tile([1, H], f32)
            nc.vector.memset(zrow[:], 0.0)
            for i in range(2):
                nc.sync.dma_start(ycomb[i][T:T + 1, :], zrow[:, :HH])

            # ---- stage transposed tokens for the router
            tokT_sb = big.tile([P, 8 * T], f32)
            for k in range(8):
                nc.sync.dma_start(tokT_sb[:, k * T:(k + 1) * T],
                                  tok_t[k * P:(k + 1) * P, :])

            # ---- routing, [token-tile partitions, expert free] layout
            maskb = big.tile([P, NT * E], bf16)
            maskf_all = big.tile([P, NT * E], f32)
            gate_all = big.tile([P, NT], f32)
            idx_all = big.tile([P, NT], f32)
            addr_i = big.tile([P, NT], i32)
            scale_all = big.tile([P, NT], f32)

            for m in range(_n("router", NT)):
                lg_ps = psr.tile([P, E], f32, name="lg_ps", tag="rps")
                for k in range(8):
                    nc.tensor.matmul(
                        lg_ps[:],
                        lhsT=tokT_sb[:, k * T + m * P: k * T + (m + 1) * P],
                        rhs=wr_sb[:, k * E:(k + 1) * E],
                        start=(k == 0), stop=(k == 7))
                lg = rtp.tile([P, E], f32)
                nc.vector.tensor_copy(lg[:], lg_ps[:])
                nrmax = rtp.tile([P, 1], f32)
                nc.vector.tensor_reduce(nrmax[:], lg[:], axis=X,
                                        op=OP.max, negate=True)
                ex = rtp.tile([P, E], f32)
                nc.scalar.activation(ex[:], lg[:], AF.Exp, bias=nrmax[:])
                esum = rtp.tile([P, 1], f32)
                nc.vector.reduce_sum(esum[:], ex[:], axis=X)
                nc.vector.reciprocal(gate_all[:, m:m + 1], esum[:])
                mf = maskf_all[:, m * E:(m + 1) * E]
                nc.vector.tensor_scalar(mf, lg[:], nrmax[:], 0.0,
                                        op0=OP.add, op1=OP.is_ge)
                nc.vector.tensor_copy(maskb[:, m * E:(m + 1) * E], mf)
                iw = rtp.tile([P, E], f32)
                nc.vector.tensor_tensor(iw[:], mf, iota_sb[:], op=OP.mult)
                nc.vector.reduce_sum(idx_all[:, m:m + 1], iw[:], axis=X)

            cum_all = big.tile([P, NT * E], f32)
            for m in range(_n("cumsum", NT)):
                cum_ps = psr.tile([P, E], f32, name="cum_ps", tag="rps")
                for k in range(m + 1):
                    nc.tensor.matmul(
                        cum_ps[:],
                        lhsT=(utri_sb[:] if k == m else ones_sb[:]),
                        rhs=maskb[:, k * E:(k + 1) * E],
                        start=(k == 0), stop=(k == m))
                nc.vector.tensor_copy(cum_all[:, m * E:(m + 1) * E], cum_ps[:])
            if "cumsum" in stg:
                # batched meta for all 8 token tiles at once
                mcum = rtp.tile([P, NT * E], f32)
                nc.vector.tensor_tensor(mcum[:], maskf_all[:, :], cum_all[:, :],
                                        op=OP.mult)
                pos = rtp.tile([P, NT], f32)
                nc.vector.reduce_sum(
                    pos[:], mcum[:].rearrange("p (m e) -> p m e", e=E), axis=X)
                nc.vector.tensor_scalar_sub(pos[:], pos[:], 1.0)
                kept = rtp.tile([P, NT], f32)
                nc.vector.tensor_scalar(kept[:], pos[:], float(CAP), None,
                                        op0=OP.is_lt)
                drop = rtp.tile([P, NT], f32)
                nc.vector.tensor_scalar(drop[:], pos[:], float(CAP), None,
                                        op0=OP.is_ge)
                addr_f = big.tile([P, NT], f32)
                nc.vector.tensor_scalar_mul(addr_f[:], idx_all[:, :], float(CAP))
                nc.vector.tensor_tensor(addr_f[:], addr_f[:], pos[:], op=OP.add)
                nc.vector.tensor_tensor(addr_f[:], addr_f[:], kept[:],
                                        op=OP.mult)
                nc.vector.tensor_scalar_mul(drop[:], drop[:], float(T))
                nc.vector.tensor_tensor(addr_f[:], addr_f[:], drop[:],
                                        op=OP.add)
                nc.vector.tensor_scalar_max(addr_f[:], addr_f[:], 0.0)
                nc.vector.tensor_scalar_min(addr_f[:], addr_f[:], float(T))
                nc.vector.tensor_copy(addr_i[:, :], addr_f[:])
                nc.vector.tensor_tensor(scale_all[:, :], gate_all[:, :],
                                        kept[:], op=OP.mult)
            else:
                addr_f = big.tile([P, NT], f32)

            # ---- dispatch scatter + AllToAll #1
            tokb_sb = big.tile([P, NT * H], bf16)
            dmask = big.tile([P, NT * T], bf16)
            for m in range(_n("dispatch", NT)):
                nc.sync.dma_start(tokb_sb[:, m * H:(m + 1) * H],
                                  tok_bf[m * P:(m + 1) * P, :])
                # dispatch one-hot: dmask[t, slot] = (addr[t] == slot)
                nc.vector.tensor_scalar(dmask[:, m * T:(m + 1) * T],
                                        siota_sb[:, :], addr_f[:, m:m + 1],
                                        None, op0=OP.is_equal)
            for n in range(2):
                for s8 in range(_n("dispatch", NT)):
                    dps = ps2.tile([P, 512], f32, name="dps", tag="dps")
                    for k in range(NT):
                        nc.tensor.matmul(
                            dps[:],
                            lhsT=dmask[:, k * T + s8 * P: k * T + (s8 + 1) * P],
                            rhs=tokb_sb[:, k * H + n * 512: k * H + (n + 1) * 512],
                            start=(k == 0), stop=(k == NT - 1))
                    xo = iop.tile([P, 512], bf16, name="xo", tag="xo")
                    nc.vector.tensor_copy(xo[:], dps[:])
                    nc.sync.dma_start(xdisp[n][s8 * P:(s8 + 1) * P, :], xo[:])
                if "dispatch" in stg:
                    if fake_collectives:
                        nc.gpsimd.dma_start(out=xrecv[n][:, :],
                                            in_=xdisp[n][:, :])
                    else:
                        nc.gpsimd.collective_compute(
                            "AllToAll", mybir.AluOpType.bypass,
                            replica_groups=RG,
                            ins=[xdisp[n][:, :].opt()],
                            outs=[xrecv[n][:, :].opt()])

            # ---- transpose received tokens (bf16 xbar transpose)
            xt_sb = big.tile([P, 8 * T], bf16)
            for k in range(_n("transpose", 8)):
                nc.sync.dma_start_transpose(
                    xt_sb[:, k * T:(k + 1) * T],
                    xrecv[k // 4][:, (k % 4) * P:(k % 4 + 1) * P])

            # ---- M1: hT[dff, slot] = relu(w1.T @ x) in bf16
            ht_sb = big.tile([P, 32 * T], bf16)
            for mb in range(_n("m1", 8)):
                w1s = w1p.tile([P, 8 * 512], bf16)
                for k in range(8):
                    nc.sync.dma_start(
                        w1s[:, k * 512:(k + 1) * 512],
                        w1[k * P:(k + 1) * P, mb * 512:(mb + 1) * 512])
                for m4 in range(4):
                    mm = mb * 4 + m4
                    for n in range(2):
                        hps = ps1.tile([P, 512], f32)
                        for k in range(8):
                            nc.tensor.matmul(
                                hps[:],
                                lhsT=w1s[:, k * 512 + m4 * P:
                                         k * 512 + (m4 + 1) * P],
                                rhs=xt_sb[:, k * T + n * 512:
                                          k * T + (n + 1) * 512],
                                start=(k == 0), stop=(k == 7))
                        nc.scalar.activation(
                            ht_sb[:, mm * T + n * 512: mm * T + (n + 1) * 512],
                            hps[:], AF.Relu)

            # ---- M2: yy[slot, h] = hT.T @ w2
            for hn in range(_n("m2", 2)):
                for tmb in range(2):
                    pss = [ps2.tile([P, 512], f32, name=f"pss{i}", tag=f"pss{i}")
                           for i in range(4)]
                    for kb in range(4):
                        w2s = w2p.tile([P, 8 * 512], bf16)
                        for k in range(8):
                            kk = kb * 8 + k
                            nc.sync.dma_start(
                                w2s[:, k * 512:(k + 1) * 512],
                                w2[kk * P:(kk + 1) * P,
                                   hn * 512:(hn + 1) * 512])
                        for t4 in range(4):
                            tm = tmb * 4 + t4
                            for k in range(8):
                                kk = kb * 8 + k
                                nc.tensor.matmul(
                                    pss[t4][:],
                                    lhsT=ht_sb[:, kk * T + tm * P:
                                               kk * T + (tm + 1) * P],
                                    rhs=w2s[:, k * 512:(k + 1) * 512],
                                    start=(kk == 0), stop=(kk == 31))
                    for t4 in range(4):
                        tm = tmb * 4 + t4
                        yo = iop.tile([P, 512], f32, name="yo", tag="yo")
                        nc.vector.tensor_copy(yo[:], pss[t4][:])
                        nc.sync.dma_start(yy[hn][tm * P:(tm + 1) * P, :],
                                          yo[:])
                # ---- AllToAll #2 + combine gather for this column half
                if fake_collectives:
                    nc.gpsimd.dma_start(out=ycomb[hn][0:T, :],
                                        in_=yy[hn][:, :])
                else:
                    nc.gpsimd.collective_compute(
                        "AllToAll", mybir.AluOpType.bypass, replica_groups=RG,
                        ins=[yy[hn][:, :].opt()],
                        outs=[ycomb[hn][0:T, :].opt()])
                for m in range(_n("combine", NT)):
                    cb = iop.tile([P, HH], f32, name="cb", tag="cb")
                    nc.gpsimd.indirect_dma_start(
                        out=cb[:], out_offset=None,
                        in_=ycomb[hn][:, :],
                        in_offset=bass.IndirectOffsetOnAxis(
                            ap=addr_i[:, m:m + 1], axis=0))
                    nc.vector.tensor_scalar_mul(cb[:], cb[:],
                                                scale_all[:, m:m + 1])
                    nc.sync.dma_start(
                        out[m * P:(m + 1) * P, hn * HH:(hn + 1) * HH], cb[:])

    nc.compile()
    return nc


def _build_and_jit():
    import jax
    from jax.sharding import Mesh, PartitionSpec
    from jax.experimental.shard_map import shard_map
    import concourse.mybir as mybir
    from concourse import bass2jax

    nc = _build_nc()

    # ---- persistent PJRT runner (adapted from bass2jax.run_bass_via_pjrt,
    # built once so repeat kernel() calls reuse the compiled executable)
    bass2jax.install_neuronx_cc_hook()
    import concourse.mybir as mb

    partition_name = (nc.partition_id_tensor.name
                      if nc.partition_id_tensor else None)
    in_names, out_names, out_avals, zero_outs = [], [], [], []
    for alloc in nc.m.functions[0].allocations:
        if not isinstance(alloc, mb.MemoryLocationSet):
            continue
        name = alloc.memorylocations[0].name
        if alloc.kind == "ExternalInput":
            if name != partition_name:
                in_names.append(name)
        elif alloc.kind == "ExternalOutput":
            shape = tuple(alloc.tensor_shape)
            dtype = mb.dt.np(alloc.dtype)
            out_names.append(name)
            out_avals.append(jax.core.ShapedArray(shape, dtype))
            zero_outs.append(np.zeros(shape, dtype))
    n_params = len(in_names)
    n_outs = len(out_avals)
    in_names_all = list(in_names) + list(out_names)
    if partition_name is not None:
        in_names_all.append(partition_name)

    def _body(*args):
        operands = list(args)
        if partition_name is not None:
            operands.append(bass2jax.partition_id_tensor())
        outs = bass2jax._bass_exec_p.bind(
            *operands,
            out_avals=tuple(out_avals),
            in_names=tuple(in_names_all),
            out_names=tuple(out_names),
            lowering_input_output_aliases=(),
            sim_require_finite=True,
            sim_require_nnan=True,
            nc=nc,
        )
        return tuple(outs)

    devices = jax.devices()[:NCORES]
    mesh = Mesh(np.asarray(devices), ("core",))
    in_specs = (PartitionSpec("core"),) * (n_params + n_outs)
    out_specs = (PartitionSpec("core"),) * n_outs
    donate = tuple(range(n_params, n_params + n_outs))
    sharded = jax.jit(
        shard_map(_body, mesh=mesh, in_specs=in_specs,
                  out_specs=out_specs, check_rep=False),
        donate_argnums=donate, keep_unused=True)

    _STATE.update(dict(
        nc=nc, sharded=sharded, in_names=in_names, out_names=out_names,
        out_avals=out_avals, zero_outs=zero_outs, mesh=mesh))
    return _STATE


def _runner():
    if "sharded" not in _STATE:
        _build_and_jit()
    return _STATE


def make_in_maps(token_inputs, w_router, w1, w2):
    """Per-core input dicts (host-side shard/layout/dtype prep only)."""
    bf = ml_dtypes.bfloat16
    ones_c = np.ones((P, P), dtype=bf)
    utri_c = np.triu(np.ones((P, P), np.float32)).astype(bf)
    iota8 = np.tile(np.arange(E, dtype=np.float32), (P, 1))
    siota = np.tile(np.arange(T, dtype=np.float32), (P, 1))
    in_maps = []
    for g in range(NCORES):
        in_maps.append({
            "tok_t": np.ascontiguousarray(token_inputs[g].T.astype(np.float32)),
            "tok_bf": np.ascontiguousarray(token_inputs[g]).astype(bf),
            "wr": np.ascontiguousarray(w_router.astype(np.float32)),
            "w1": np.ascontiguousarray(w1[g]).astype(bf),
            "w2": np.ascontiguousarray(w2[g]).astype(bf),
            "ones_c": ones_c,
            "utri_c": utri_c,
            "iota8": iota8,
            "siota": siota,
        })
    return in_maps


def run_in_maps(in_maps):
    st = _runner()
    concat_in = [
        np.concatenate([np.asarray(in_maps[c][name])
                        for c in range(NCORES)], axis=0)
        for name in st["in_names"]
    ]
    concat_zeros = [np.zeros((NCORES * z.shape[0], *z.shape[1:]), z.dtype)
                    for z in st["zero_outs"]]
    out_arrs = st["sharded"](*concat_in, *concat_zeros)
    res = []
    for c in range(NCORES):
        res.append({
            name: np.asarray(out_arrs[i]).reshape(
                NCORES, *st["out_avals"][i].shape)[c]
            for i, name in enumerate(st["out_names"])
        })
    return res


def kernel(token_inputs, w_router, w1, w2, expert_capacity):
    token_inputs = np.asarray(token_inputs)
    w_router = np.asarray(w_router)
    w1 = np.asarray(w1)
    w2 = np.asarray(w2)
    assert int(expert_capacity) == CAP
    assert token_inputs.shape == (G, T, H)
    in_maps = make_in_maps(token_inputs, w_router, w1, w2)
    try:
        res = run_in_maps(in_maps)
    except Exception:
        # fallback: stock SPMD runner (recompiles per call, but robust)
        from concourse import bass_utils
        nc = _STATE.get("nc") or _build_nc()
        res = bass_utils.run_bass_kernel_spmd(
            nc, in_maps, core_ids=list(range(NCORES))).results
    return np.stack([res[g]["out"] for g in range(NCORES)], axis=0)

